# revision 38
# baseline (speedup 1.0000x reference)
"""RENBase simulate_sequence kernel for Trainium2 (8 NeuronCores, Bass/Tile).

Math per timestep t (see reference):
    b  = C1 @ x + D12 @ u_t + bv                  # [nv, batch] feature-major
    w  = solve  w = relu(D11 @ w + b)             # D11 strictly lower-triangular
    x' = A @ x + B1 @ w + B2 @ u_t + bx
    y_t= C2 @ x + D21 @ w + D22 @ u_t + by

The triangular relu solve is computed by blocked fixed-point iteration:
D11 is split into two 128-feature blocks; within each block
w <- relu(D_blk @ w + c) converges superlinearly (the block is nilpotent and
a contraction).  K iterations per block drive the truncation error far below
the fp32 rounding floor (validated on the real inputs: single-solve error
7e-7 @ k=8, 6e-9 @ k=10; the end-to-end difference vs an exact solver is
then dominated by fp32 rounding-path noise, which the chaotic closed loop
amplifies to ~1e-3 for ANY fp32 implementation).

Everything is kept feature-major on device ([feature partitions, batch free])
so every matmul's stationary operand is a constant parameter transpose and no
on-device transposes are needed; the host transposes inputs/outputs instead.

The whole u sequence is loaded into SBUF once (8 MB) and y accumulates into
an SBUF ring flushed in T/YCH chunk DMAs — there are no per-timestep DMAs,
which keeps every instruction within its ISA sync-wait slot budget.

Sharding: data-parallel over batch (512 = 8 cores x 64), parameters
replicated; the time scan and the solve stay sequential per core.
"""

import numpy as np

import bass_rust
import concourse.bass as bass
import concourse.tile as tile
from concourse import mybir
from concourse.bass_utils import run_bass_kernel_spmd

T, B = 512, 512
NU, NX, NV, NY = 64, 128, 256, 64
NCORES = 8
BL = B // NCORES  # batch per core = 64
H = 128           # solve block size (NV = 2*H)

K0 = 10  # fixed-point iterations, block 0 (incl. the initial w = relu(b))
K1 = 10  # fixed-point iterations, block 1
YCH = 128  # timesteps per y output chunk DMA (4 chunks + 4 other DMAs = 8
           # total = one per DMA-sem lane, so no lane is ever reused)

F32 = mybir.dt.float32
_Y_DMA_GPSIMD = False  # SWDGE path hangs with >1 chunk DMA; SP lanes suffice

# all weights packed into one [128, _PK_COLS] f32 tensor, one DMA, one sem.
# name -> (col offset, n partitions, n cols)
_PK_LAYOUT = {}
_PK_COLS = 0
for _name, _p, _c in [
    ("C1T0", NX, H), ("C1T1", NX, H),
    ("D12T0", NU, H), ("D12T1", NU, H),
    ("D00T", H, H), ("D10T", H, H), ("DbbT", H, H),
    ("AT", NX, NX), ("B1T0", H, NX), ("B1T1", H, NX), ("B2T", NU, NX),
    ("C2T", NX, NY), ("D21T0", H, NY), ("D21T1", H, NY), ("D22T", NU, NY),
    ("IDT", H, H),
    ("bv0", H, 1), ("bv1", H, 1), ("bx", NX, 1), ("by", NY, 1),
]:
    _PK_LAYOUT[_name] = (_PK_COLS, _p, _c)
    _PK_COLS += _c


def _build_program(Tn=T, ych=YCH):
    from contextlib import ExitStack

    nc = bass.Bass()

    x0T = nc.declare_dram_parameter("x0T", [NX, BL], F32, isOutput=False)
    uT = nc.declare_dram_parameter("uT", [Tn * NU, BL], F32, isOutput=False)
    wpk_d = nc.declare_dram_parameter("WPK", [128, _PK_COLS], F32, isOutput=False)
    yT = nc.declare_dram_parameter("yT", [Tn * NY, BL], F32, isOutput=True)
    x1T = nc.declare_dram_parameter("x1T", [NX, BL], F32, isOutput=True)
    xmid = nc.dram_tensor("xmid", [NX, BL], F32)

    # The ISA sync-wait immediate is 14 bits; per-proc sem ticks must stay
    # under 16384.  PE ticks ~50/timestep, so a single TileContext tops out
    # around T=300.  Split the scan into sequential TileContext epochs —
    # each gets fresh semaphores (values restart at 0), x is handed across
    # via DRAM, u is (re)loaded per epoch.
    TE = min(Tn, 256)
    n_ep = Tn // TE
    assert Tn == TE * n_ep
    ych = min(ych, TE)

    for ep in range(n_ep):
        _build_epoch(
            nc, ep, n_ep, TE, ych,
            x0T if ep == 0 else xmid,
            x1T if ep == n_ep - 1 else xmid,
            uT, wpk_d, yT,
        )

    _fix_sync_waits(nc)
    return nc


def _build_epoch(nc, ep, n_ep, TE, ych, x_in, x_out, uT, wpk_d, yT):
    from contextlib import ExitStack

    Ident = mybir.ActivationFunctionType.Identity

    with tile.TileContext(nc) as tc, ExitStack() as ctx:
        consts = ctx.enter_context(tc.tile_pool(name="consts", bufs=1))
        xpool = ctx.enter_context(tc.tile_pool(name="x", bufs=1))
        bpool = ctx.enter_context(tc.tile_pool(name="b", bufs=2))
        wpool = ctx.enter_context(tc.tile_pool(name="w", bufs=4))
        yring = ctx.enter_context(tc.tile_pool(name="yring", bufs=1))
        bps = ctx.enter_context(tc.tile_pool(name="bps", bufs=2, space="PSUM"))
        iterps = ctx.enter_context(tc.tile_pool(name="iterps", bufs=2, space="PSUM"))
        xps = ctx.enter_context(tc.tile_pool(name="xps", bufs=2, space="PSUM"))
        yps = ctx.enter_context(tc.tile_pool(name="yps", bufs=2, space="PSUM"))

        wpk = consts.tile([128, _PK_COLS], F32, tag="wpk", name="wpk")
        nc.sync.dma_start(wpk[:], wpk_d[:])
        cw = {
            name: wpk[0:p, off : off + c] for name, (off, p, c) in _PK_LAYOUT.items()
        }

        # this epoch's u slice resident in SBUF: [nu, t, b]
        u_all = consts.tile([NU, TE, BL], F32, tag="u_all", name="u_all")
        nc.sync.dma_start(
            u_all[:],
            uT.rearrange("(t q) b -> q t b", q=NU)[:, ep * TE : (ep + 1) * TE, :],
        )

        xbuf = [
            xpool.tile([NX, BL], F32, tag=f"x{i}", name=f"x{i}") for i in range(2)
        ]
        nc.sync.dma_start(xbuf[0][:], x_in[:])

        # Warm-up reads on PE and ACT: absorb the one-time DMA-queue waits
        # for wpk/u_all/x0 on each consuming engine so the first real
        # matmul / activation stays within its ISA sync-wait slot budget
        # (matmul holds one wait command).  Each warm-up is a host slot the
        # sync-wait spill pass below can also use.
        warm_sb = consts.tile([1, 16], F32, tag="warm_sb", name="warm_sb")
        w00 = wpk[0:1, 0:1]
        ua0 = u_all[0:1, 0:1, 0:1].rearrange("a b c -> a (b c)")
        pe_warm_srcs = [w00, ua0, xbuf[0][0:1, 0:1], w00, w00]
        for i, src in enumerate(pe_warm_srcs):
            warm_ps = bps.tile([1, 1], F32, tag="bps", name="warm_ps")
            nc.tensor.matmul(
                warm_ps[0:1, 0:1], w00 if i == 2 else src,
                src if i == 2 else w00, start=True, stop=True,
            )
        for i, src in enumerate([w00, ua0, xbuf[0][0:1, 0:1], w00]):
            nc.scalar.copy(warm_sb[0:1, i : i + 1], src)
        warm_sb2 = consts.tile([1, 4], F32, tag="warm_sb2", name="warm_sb2")
        for i in range(2):
            nc.vector.tensor_scalar_max(warm_sb2[0:1, i : i + 1], w00, 0.0)

        def timestep(t, cur, nxt, y_out):
            u_sb = u_all[:, t, :]

            # b = C1 @ x + D12 @ u; +bv applied on both read paths
            b_ps, b_sb = [], []
            for g, (c1t, d12t, bvg) in enumerate(
                (("C1T0", "D12T0", "bv0"), ("C1T1", "D12T1", "bv1"))
            ):
                ps = bps.tile([H, BL], F32, tag="bps")
                nc.tensor.matmul(ps[:], cw[c1t], cur[:], start=True, stop=False)
                nc.tensor.matmul(ps[:], cw[d12t], u_sb, start=False, stop=True)
                sb = bpool.tile([H, BL], F32, tag=f"b{g}")
                nc.scalar.activation(sb[:], ps[:], Ident, bias=cw[bvg])
                b_ps.append(ps)
                b_sb.append(sb)

            # x' and y accumulations that do not need w: issue early
            x_ps = xps.tile([NX, BL], F32, tag="xps")
            nc.tensor.matmul(x_ps[:], cw["AT"], cur[:], start=True, stop=False)
            nc.tensor.matmul(x_ps[:], cw["B2T"], u_sb, start=False, stop=False)
            y_ps = yps.tile([NY, BL], F32, tag="yps")
            nc.tensor.matmul(y_ps[:], cw["C2T"], cur[:], start=True, stop=False)
            nc.tensor.matmul(y_ps[:], cw["D22T"], u_sb, start=False, stop=False)

            # blocked fixed-point solve of w = relu(D11 w + b).
            # The initial w = relu(c) reads the PSUM directly (bias fused in
            # the DVE op) so the SBUF copy of c is only ever read by PE —
            # keeps every instruction at a single sync wait.
            def solve_block(dT, init_ps, init_bias, c_sb, k, tag):
                w = wpool.tile([H, BL], F32, tag=tag, name=tag)
                if init_bias is None:
                    nc.vector.tensor_scalar_max(w[:], init_ps[:], 0.0)
                else:
                    nc.vector.tensor_scalar(
                        w[:], init_ps[:], init_bias, 0.0,
                        mybir.AluOpType.add, mybir.AluOpType.max,
                    )
                for _ in range(k - 1):
                    ps = iterps.tile([H, BL], F32, tag="iterps", name="iterps")
                    nc.tensor.matmul(ps[:], cw[dT], w[:], start=True, stop=False)
                    nc.tensor.matmul(ps[:], cw["IDT"], c_sb[:], start=False, stop=True)
                    w = wpool.tile([H, BL], F32, tag=tag, name=tag)
                    nc.vector.tensor_scalar_max(w[:], ps[:], 0.0)
                return w

            w0 = solve_block("D00T", b_ps[0], cw["bv0"], b_sb[0], K0, "w0")

            # c1 = b1 + D10 @ w0
            c_ps = iterps.tile([H, BL], F32, tag="iterps", name="iterps")
            nc.tensor.matmul(c_ps[:], cw["D10T"], w0[:], start=True, stop=False)
            nc.tensor.matmul(c_ps[:], cw["IDT"], b_sb[1][:], start=False, stop=True)
            c1 = bpool.tile([H, BL], F32, tag="c1")
            nc.scalar.copy(c1[:], c_ps[:])

            w1 = solve_block("DbbT", c_ps, None, c1, K1, "w1")

            # x' = A x + B2 u + B1 w (+ bx)
            nc.tensor.matmul(x_ps[:], cw["B1T0"], w0[:], start=False, stop=False)
            nc.tensor.matmul(x_ps[:], cw["B1T1"], w1[:], start=False, stop=True)
            nc.scalar.activation(nxt[:], x_ps[:], Ident, bias=cw["bx"])

            # y = C2 x + D22 u + D21 w (+ by), straight into the y ring slice
            nc.tensor.matmul(y_ps[:], cw["D21T0"], w0[:], start=False, stop=False)
            nc.tensor.matmul(y_ps[:], cw["D21T1"], w1[:], start=False, stop=True)
            nc.scalar.activation(y_out[:], y_ps[:], Ident, bias=cw["by"])

        for c in range(TE // ych):
            yc = yring.tile([NY, ych, BL], F32, tag="yring", name="yring")
            # fence: first touch of the recycled ring slot happens on this
            # throwaway copy, which absorbs the slot's DMA-read wait so the
            # real y evictions keep a single sync wait each.
            nc.scalar.copy(yc[0:1, 0, 0:1], w00)
            for j in range(ych):
                t = c * ych + j
                timestep(t, xbuf[t % 2], xbuf[(t + 1) % 2], yc[:, j, :])
            dma_engine = nc.gpsimd if _Y_DMA_GPSIMD else nc.sync
            dma_engine.dma_start(
                yT.rearrange("(t p) b -> p t b", p=NY)[
                    :, ep * TE + c * ych : ep * TE + (c + 1) * ych, :
                ],
                yc[:],
            )

        nc.sync.dma_start(x_out[:], xbuf[TE % 2][:])


def _fix_sync_waits(nc):
    """Walrus rejects instructions whose sync_info carries more wait commands
    than the instruction encoding has slots for.  Tile emits duplicate waits
    on the same semaphore and occasionally one wait too many.  Dedupe every
    instruction; spill remaining excess onto earlier same-engine instructions
    with spare slots (the block order is a topological order of the dep
    graph, so hosting a wait after its producing instruction cannot deadlock,
    and same-engine in-order issue preserves the gating).  The kernel-tail
    drain instead forwards its excess onto the end-of-program barrier
    EventSemaphores that follow it."""

    def wait_cap(inst):
        # ISA sync budget: EventSemaphore holds 2 wait commands, every other
        # instruction exactly 1 (walrus setupSyncWait rejects more).
        return 2 if inst.__class__.__name__ == "InstEventSemaphore" else 1

    # Engine-proc sem prefix for strictly in-order engines: a wait on your
    # own engine's tick sem is subsumed by in-order issue+completion
    # (PE matmuls complete in pc order; ACT/DVE are single strict-FIFO
    # pipelines with a drain after every op; SP is the sequencer itself).
    own_proc_prefix = {
        mybir.EngineType.PE: "PE_",
        mybir.EngineType.Activation: "Activation_",
        mybir.EngineType.DVE: "DVE_",
        mybir.EngineType.SP: "SP_",
    }

    fn = nc.m.functions[0]
    for bb in fn.blocks:
        insts = list(bb.instructions)

        for inst in insts:
            si = inst.sync_info
            if si is None or not si.on_wait:
                continue
            ws = []
            bass_rust.merge_waits(ws, list(si.on_wait))
            pfx = own_proc_prefix.get(inst.engine)
            if pfx is not None:
                ws = [w for w in ws if not w.ant_name.startswith(pfx)]
            if len(ws) < len(si.on_wait):
                inst.sync_info = bass_rust.SyncInfo(
                    on_wait=ws, on_update=list(si.on_update)
                )

        cum = {}       # sem id -> cumulative update value so far (this bb)
        sem_hist = {}  # sem id -> list of (pos, cum_after)
        hosts = {}     # engine -> list of [pos, inst, waits(list)]

        def producer_pos(sem_id, value):
            for pos, cv in sem_hist.get(sem_id, ()):
                if cv >= value:
                    return pos
            return -1  # produced in an earlier block

        def set_waits(hinst, hws):
            hsi = hinst.sync_info
            hinst.sync_info = bass_rust.SyncInfo(
                on_wait=hws,
                on_update=list(hsi.on_update) if hsi is not None else [],
            )

        hostable = ("InstMatmult", "InstActivation", "InstTensorScalarPtr",
                    "InstDMACopy", "InstNoOp", "InstTensorCopy",
                    "InstTensorReduce", "InstTensorTensor")

        for pos, inst in enumerate(insts):
            si = inst.sync_info
            ws = list(si.on_wait) if si is not None and si.on_wait else []
            cap = wait_cap(inst)
            if len(ws) > cap and inst.__class__.__name__ != "InstDrain":
                cands = hosts.get(inst.engine, [])
                for w in list(ws):
                    if len(ws) <= cap:
                        break
                    pp = producer_pos(w.id, w.wait_value)
                    done = False
                    # 1) merge: bump a host already waiting on the same sem
                    for ci in range(len(cands) - 1, -1, -1):
                        hpos, hinst, hws = cands[ci]
                        if hpos <= pp:
                            continue
                        for wi, hw in enumerate(hws):
                            if hw.id == w.id:
                                if hw.wait_value < w.wait_value:
                                    hws[wi] = w
                                    set_waits(hinst, list(hws))
                                ws.remove(w)
                                done = True
                                break
                        if done:
                            break
                    if done:
                        continue
                    # 2) spill: host with a free slot
                    for ci in range(len(cands) - 1, -1, -1):
                        hpos, hinst, hws = cands[ci]
                        if hpos <= pp or len(hws) >= wait_cap(hinst):
                            continue
                        hws.append(w)
                        set_waits(hinst, list(hws))
                        ws.remove(w)
                        done = True
                        break
                assert len(ws) <= cap, (
                    f"cannot re-home {len(ws) - cap} waits of {inst.name} "
                    f"({inst.__class__.__name__}, cap {cap}): "
                    f"{[(w.ant_name, w.wait_value) for w in ws]}"
                )
                inst.sync_info = bass_rust.SyncInfo(
                    on_wait=ws, on_update=list(si.on_update)
                )
            if inst.__class__.__name__ in hostable:
                hosts.setdefault(inst.engine, []).append([pos, inst, ws])
                if len(hosts[inst.engine]) > 96:
                    hosts[inst.engine] = hosts[inst.engine][-96:]
            if si is not None and si.on_update:
                for up in si.on_update:
                    cum[up.id] = cum.get(up.id, 0) + (up.update_value or 1)
                    sem_hist.setdefault(up.id, []).append((pos, cum[up.id]))

    # context-tail drains: forward their excess onto the barrier
    # EventSemaphores that follow them
    for end_bb in fn.blocks:
        _rehome_drains(end_bb)


def _rehome_drains(end_bb):
    insts = list(end_bb.instructions)
    for idx, inst in enumerate(insts):
        if inst.__class__.__name__ != "InstDrain":
            continue
        si = inst.sync_info
        if si is None or not si.on_wait or len(si.on_wait) <= 1:
            continue
        excess = list(si.on_wait)[1:]
        inst.sync_info = bass_rust.SyncInfo(
            on_wait=list(si.on_wait)[:1], on_update=list(si.on_update)
        )
        for host in insts[idx + 1 :]:
            if not excess:
                break
            if host.__class__.__name__ != "InstEventSemaphore":
                continue
            hsi = host.sync_info
            hws = list(hsi.on_wait) if hsi is not None and hsi.on_wait else []
            room = 2 - len(hws)
            if room <= 0:
                continue
            hws.extend(excess[:room])
            excess = excess[room:]
            host.sync_info = bass_rust.SyncInfo(
                on_wait=hws,
                on_update=list(hsi.on_update) if hsi is not None else [],
            )
        assert not excess, f"could not re-home {len(excess)} drain waits"


def _host_prep(inputs):
    f = np.float32
    A = np.asarray(inputs["A"], f)
    B1 = np.asarray(inputs["B1"], f)
    B2 = np.asarray(inputs["B2"], f)
    C1 = np.asarray(inputs["C1"], f)
    C2 = np.asarray(inputs["C2"], f)
    D11 = np.asarray(inputs["D11"], f)
    D12 = np.asarray(inputs["D12"], f)
    D21 = np.asarray(inputs["D21"], f)
    D22 = np.asarray(inputs["D22"], f)
    bx = np.asarray(inputs["bx"], f)
    bv = np.asarray(inputs["bv"], f)
    by = np.asarray(inputs["by"], f)

    mats = {
        "C1T0": C1[:H, :].T, "C1T1": C1[H:, :].T,
        "D12T0": D12[:H, :].T, "D12T1": D12[H:, :].T,
        "D00T": D11[:H, :H].T, "D10T": D11[H:, :H].T, "DbbT": D11[H:, H:].T,
        "AT": A.T, "B1T0": B1[:, :H].T, "B1T1": B1[:, H:].T, "B2T": B2.T,
        "C2T": C2.T, "D21T0": D21[:, :H].T, "D21T1": D21[:, H:].T, "D22T": D22.T,
        "IDT": np.eye(H, dtype=f),
        "bv0": bv[:H].reshape(H, 1), "bv1": bv[H:].reshape(H, 1),
        "bx": bx.reshape(NX, 1), "by": by.reshape(NY, 1),
    }
    wpk = np.zeros((128, _PK_COLS), f)
    for name, (off, p, c) in _PK_LAYOUT.items():
        m = np.asarray(mats[name], f)
        assert m.shape == (p, c), (name, m.shape, (p, c))
        wpk[:p, off : off + c] = m
    return wpk


def _in_maps(inputs):
    f = np.float32
    x0 = np.asarray(inputs["x0"], f)
    u = np.asarray(inputs["u"], f)
    wpk = _host_prep(inputs)
    maps = []
    for c in range(NCORES):
        lo, hi = c * BL, (c + 1) * BL
        x0T = np.ascontiguousarray(x0[lo:hi, :].T)                # [NX, BL]
        uT = np.ascontiguousarray(
            u[:, lo:hi, :].transpose(0, 2, 1).reshape(T * NU, BL)
        )
        maps.append({"x0T": x0T, "uT": uT, "WPK": wpk})
    return maps


def kernel(**inputs):
    nc = _build_program()
    maps = _in_maps(inputs)
    res = run_bass_kernel_spmd(nc, maps, list(range(NCORES)))
    x1 = np.empty((B, NX), np.float32)
    y = np.empty((T, B, NY), np.float32)
    for c in range(NCORES):
        lo, hi = c * BL, (c + 1) * BL
        out = res.results[c]
        x1[lo:hi, :] = out["x1T"].T
        y[:, lo:hi, :] = out["yT"].reshape(T, NY, BL).transpose(0, 2, 1)
    return x1, y
